# revision 5
# baseline (speedup 1.0000x reference)
"""Trainium2 Bass kernel for fused single-head attention.

Reference (jax, fp32):
    qkv = x @ W_qkv + b_qkv            # [B,T,C] @ [C,3C]
    q, k, v = split(qkv, 3, axis=-1)
    att = softmax(q @ k.T / sqrt(C))   # per batch, [T,T]
    out = att @ v                      # [B,T,C]

Shapes: B=4, T=2048, C=1024, fp32 in/out.

Sharding (8 cores, pure SPMD, no collectives): core c handles batch
b = c//2 and query-row half h = c%2 (1024 query rows). Each core
computes K and V for the full T of its batch (K/V projection is
duplicated between the two cores of a batch; costs ~20% extra PE time
but avoids all cross-core communication).

Per-core layout trick: the kernel program must be identical on all
cores (SPMD), so instead of slicing a different query-half per core,
the host rolls the T axis of x so that the core's own 1024 query rows
always come first. Attention is invariant to a permutation of the
key/value axis, so K/V over the rolled axis give the same result.

On-device dataflow (all matmuls bf16 operands, fp32 PSUM accumulate):
  xT  [C, T]   (transposed by host, bf16)
  QT  [c, tq] = Wq.T-stationary @ xT      (+bq, kept transposed)
  KT  [c, s]  = Wk.T-stationary @ xT      (+bk)
  V   [s, c]  = xT-stationary @ Wv        (no bias; folded in at the end)
  SCT [s, tq] = KT-stationary @ QT        (scores, transposed)
  attT[s, tq] = exp(SCT / 32)             (no max-subtraction: logits are
                                           O(3) by construction, exp is safe)
  U   [tq, c] = attT-stationary @ V       (unnormalized output)
  S   [tq, 1] = attT-stationary @ ones    (softmax denominators)
  out = U * (1/S) + bv                    (softmax normalization + v-bias:
                                           att/S rows sum to 1 exactly, so
                                           adding bv at the end is exact)
"""

from contextlib import ExitStack

import ml_dtypes
import numpy as np

import concourse.tile as tile
from concourse import bacc, mybir
from concourse.bass_utils import run_bass_kernel_spmd

B, T, C = 4, 2048, 1024
N_CORES = 8
TQ = T // 2          # query rows per core
CT = C // 128        # 8 c-tiles
ST = T // 128        # 16 s-tiles
BF16 = mybir.dt.bfloat16
F32 = mybir.dt.float32

_NC_CACHE = None


def _build_nc(repeats=1):
    """repeats>1 re-emits the whole computation R times back-to-back in one
    NEFF — used by the bench harness to measure device time differentially
    (dispatch overhead cancels in t(R) - t(1))."""
    nc = bacc.Bacc("TRN2", target_bir_lowering=False, debug=False, num_devices=N_CORES)

    xT = nc.declare_dram_parameter("xT", [C, T], BF16, isOutput=False)
    W = nc.declare_dram_parameter("W", [C, 3 * C], BF16, isOutput=False)
    bq = nc.declare_dram_parameter("bq", [128, CT], F32, isOutput=False)
    bk = nc.declare_dram_parameter("bk", [128, CT], F32, isOutput=False)
    bvr = nc.declare_dram_parameter("bvr", [128, C], F32, isOutput=False)
    out = nc.declare_dram_parameter("out", [TQ, C], F32, isOutput=True)

    with ExitStack() as ctx:
        tc = ctx.enter_context(tile.TileContext(nc))
        singles = ctx.enter_context(tc.tile_pool(name="singles", bufs=1))
        wpool = ctx.enter_context(tc.tile_pool(name="wpool", bufs=2))
        opool = ctx.enter_context(tc.tile_pool(name="opool", bufs=2))
        small = ctx.enter_context(tc.tile_pool(name="small", bufs=2))
        psum = ctx.enter_context(tc.tile_pool(name="psum", bufs=3, space="PSUM"))

        # ---- resident inputs -------------------------------------------------
        xT_sb = singles.tile([128, CT, T], BF16)  # [p, j, t] = xT[j*128+p, t]
        for j in range(CT):
            nc.sync.dma_start(out=xT_sb[:, j, :], in_=xT[j * 128 : (j + 1) * 128, :])
        bq_sb = singles.tile([128, CT], F32)
        nc.sync.dma_start(out=bq_sb, in_=bq[:, :])
        bk_sb = singles.tile([128, CT], F32)
        nc.sync.dma_start(out=bk_sb, in_=bk[:, :])
        bvr_sb = singles.tile([128, C], F32)
        nc.sync.dma_start(out=bvr_sb, in_=bvr[:, :])
        ones_sb = singles.tile([128, 1], BF16)
        nc.vector.memset(ones_sb, 1.0)

        QT_sb = singles.tile([128, CT, TQ], BF16)
        KT_sb = singles.tile([128, CT, T], BF16)
        V_sb = singles.tile([128, ST, C], BF16)
        attT_sb = singles.tile([128, ST, TQ], BF16)

        def load_w_third(col0):
            w_sb = wpool.tile([128, CT, C], BF16, tag="w")
            for j in range(CT):
                nc.sync.dma_start(
                    out=w_sb[:, j, :],
                    in_=W[j * 128 : (j + 1) * 128, col0 : col0 + C],
                )
            return w_sb

        def emit_proj():
            # Q projection: QT[c, tq]
            wq_sb = load_w_third(0)
            for m in range(CT):
                for q2 in range(TQ // 512):
                    ps = psum.tile([128, 512], F32, tag="mm")
                    for j in range(CT):
                        nc.tensor.matmul(
                            ps,
                            wq_sb[:, j, m * 128 : (m + 1) * 128],
                            xT_sb[:, j, q2 * 512 : (q2 + 1) * 512],
                            start=(j == 0),
                            stop=(j == CT - 1),
                        )
                    nc.vector.tensor_scalar_add(
                        QT_sb[:, m, q2 * 512 : (q2 + 1) * 512], ps, bq_sb[:, m : m + 1]
                    )
            # K projection: KT[c, s]
            wk_sb = load_w_third(C)
            for m in range(CT):
                for s2 in range(T // 512):
                    ps = psum.tile([128, 512], F32, tag="mm")
                    for j in range(CT):
                        nc.tensor.matmul(
                            ps,
                            wk_sb[:, j, m * 128 : (m + 1) * 128],
                            xT_sb[:, j, s2 * 512 : (s2 + 1) * 512],
                            start=(j == 0),
                            stop=(j == CT - 1),
                        )
                    nc.vector.tensor_scalar_add(
                        KT_sb[:, m, s2 * 512 : (s2 + 1) * 512], ps, bk_sb[:, m : m + 1]
                    )
            # V projection: V[s, c] (x-stationary)
            wv_sb = load_w_third(2 * C)
            for st in range(ST):
                for c2 in range(C // 512):
                    ps = psum.tile([128, 512], F32, tag="mm")
                    for j in range(CT):
                        nc.tensor.matmul(
                            ps,
                            xT_sb[:, j, st * 128 : (st + 1) * 128],
                            wv_sb[:, j, c2 * 512 : (c2 + 1) * 512],
                            start=(j == 0),
                            stop=(j == CT - 1),
                        )
                    nc.scalar.activation(
                        out=V_sb[:, st, c2 * 512 : (c2 + 1) * 512],
                        in_=ps,
                        func=mybir.ActivationFunctionType.Copy,
                    )

        def emit_scores():
            # scores (transposed) + exp: attT[s, tq] = exp(KT.T-strips @ QT / 32)
            for q2 in range(TQ // 512):
                for st in range(ST):
                    ps = psum.tile([128, 512], F32, tag="mm")
                    for m in range(CT):
                        nc.tensor.matmul(
                            ps,
                            KT_sb[:, m, st * 128 : (st + 1) * 128],
                            QT_sb[:, m, q2 * 512 : (q2 + 1) * 512],
                            start=(m == 0),
                            stop=(m == CT - 1),
                        )
                    nc.scalar.activation(
                        out=attT_sb[:, st, q2 * 512 : (q2 + 1) * 512],
                        in_=ps,
                        func=mybir.ActivationFunctionType.Exp,
                        scale=1.0 / 32.0,
                    )

        def emit_av():
            # AV + denominators + normalize + v-bias + store
            for i in range(TQ // 128):
                av = psum.tile([128, C], F32, tag="av", bufs=2)
                sS = psum.tile([128, 1], F32, tag="s", bufs=1)
                for st in range(ST):
                    lhsT = attT_sb[:, st, i * 128 : (i + 1) * 128]
                    nc.tensor.matmul(
                        av[:, 0:512],
                        lhsT,
                        V_sb[:, st, 0:512],
                        start=(st == 0),
                        stop=(st == ST - 1),
                    )
                    nc.tensor.matmul(
                        av[:, 512:1024],
                        lhsT,
                        V_sb[:, st, 512:1024],
                        start=(st == 0),
                        stop=(st == ST - 1),
                    )
                    nc.tensor.matmul(
                        sS, lhsT, ones_sb, start=(st == 0), stop=(st == ST - 1)
                    )
                recip = small.tile([128, 1], F32, tag="recip")
                nc.vector.reciprocal(recip, sS)
                o_sb = opool.tile([128, C], F32, tag="o")
                nc.vector.tensor_scalar_mul(o_sb[:, 0:512], av[:, 0:512], recip)
                nc.vector.tensor_scalar_mul(o_sb[:, 512:1024], av[:, 512:1024], recip)
                nc.vector.tensor_add(o_sb, o_sb, bvr_sb)
                nc.sync.dma_start(out=out[i * 128 : (i + 1) * 128, :], in_=o_sb)

        for _ in range(repeats):
            emit_proj()
            emit_scores()
            emit_av()

    nc.compile()
    return nc


def _get_nc():
    global _NC_CACHE
    if _NC_CACHE is None:
        _NC_CACHE = _build_nc()
    return _NC_CACHE


def _make_in_maps(x, W_qkv, b_qkv):
    W_b16 = np.ascontiguousarray(W_qkv.astype(ml_dtypes.bfloat16))
    # biases laid out [128, CT]: col j holds bias[j*128 : (j+1)*128]
    bq = np.ascontiguousarray(b_qkv[:C].reshape(CT, 128).T.astype(np.float32))
    bk = np.ascontiguousarray(b_qkv[C : 2 * C].reshape(CT, 128).T.astype(np.float32))
    bvr = np.ascontiguousarray(
        np.broadcast_to(b_qkv[2 * C :].astype(np.float32), (128, C))
    )
    in_maps = []
    for core in range(N_CORES):
        b, h = core // 2, core % 2
        xb = x[b]
        if h == 1:  # roll so this core's query half comes first
            xb = np.concatenate([xb[TQ:], xb[:TQ]], axis=0)
        xT = np.ascontiguousarray(xb.T).astype(ml_dtypes.bfloat16)
        in_maps.append({"xT": xT, "W": W_b16, "bq": bq, "bk": bk, "bvr": bvr})
    return in_maps


def _run(x, W_qkv, b_qkv, trace=False, **spmd_kwargs):
    nc = _get_nc()
    in_maps = _make_in_maps(x, W_qkv, b_qkv)
    res = run_bass_kernel_spmd(
        nc, in_maps, list(range(N_CORES)), trace=trace, **spmd_kwargs
    )
    out = np.empty((B, T, C), dtype=np.float32)
    for core in range(N_CORES):
        b, h = core // 2, core % 2
        out[b, h * TQ : (h + 1) * TQ, :] = res.results[core]["out"]
    return out, res


def kernel(x, W_qkv, b_qkv):
    x = np.asarray(x)
    W_qkv = np.asarray(W_qkv)
    b_qkv = np.asarray(b_qkv)
    out, _ = _run(x, W_qkv, b_qkv)
    return out


# revision 7
# speedup vs baseline: 1.1366x; 1.1366x over previous
"""Trainium2 Bass kernel for fused single-head attention.

Reference (jax, fp32):
    qkv = x @ W_qkv + b_qkv            # [B,T,C] @ [C,3C]
    q, k, v = split(qkv, 3, axis=-1)
    att = softmax(q @ k.T / sqrt(C))   # per batch, [T,T]
    out = att @ v                      # [B,T,C]

Shapes: B=4, T=2048, C=1024, fp32 in/out.

Sharding (8 cores, pure SPMD): core c handles batch b = c//2 and
query-row half h = c%2 (1024 query rows). Each core projects K and V
for only its own 1024 rows; the halves are exchanged between the two
cores of a batch with a pairwise AllGather (groups [0,1],[2,3],...),
so no projection work is duplicated. The K exchange is issued right
after the K projection so it overlaps the Q and V projections on the
PE; the V exchange overlaps the score matmuls.

Per-core layout trick: the kernel program must be identical on all
cores (SPMD), so instead of slicing a different query-half per core,
the host rolls the T axis of x so that the core's own 1024 query rows
always come first. Attention is invariant to a permutation of the
key/value axis; after the exchange both cores of a pair use the same
pair-global order [even-core half, odd-core half] for K/V/att.

On-device dataflow (all matmuls bf16 operands, fp32 PSUM accumulate):
  xT  [C, T]   (transposed by host, bf16)
  KT-own [c, 1024] = Wk.T-stationary @ xT[:, :1024]  (+bk)  → AllGather
  QT  [c, tq] = Wq.T-stationary @ xT[:, :1024]       (+bq, kept transposed)
  V-own  [1024, c] = xT-stationary @ Wv              (no bias) → AllGather
  SCT [s, tq] = KT-stationary @ QT        (scores, transposed)
  attT[s, tq] = exp(SCT / 32)             (no max-subtraction: logits are
                                           O(3) by construction, exp is safe)
  U   [tq, c] = attT-stationary @ V       (unnormalized output)
  S   [tq, 1] = attT-stationary @ ones    (softmax denominators)
  out = U * (1/S) + bv                    (softmax normalization + v-bias:
                                           att/S rows sum to 1 exactly, so
                                           adding bv at the end is exact)
"""

from contextlib import ExitStack

import ml_dtypes
import numpy as np

import concourse.tile as tile
from concourse import bacc, mybir
from concourse.bass_utils import run_bass_kernel_spmd

B, T, C = 4, 2048, 1024
N_CORES = 8
TQ = T // 2          # query rows per core
CT = C // 128        # 8 c-tiles
ST = T // 128        # 16 s-tiles
STH = ST // 2        # own-half s-tiles
BF16 = mybir.dt.bfloat16
F32 = mybir.dt.float32
PAIRS = [[0, 1], [2, 3], [4, 5], [6, 7]]

_NC_CACHE = None


def _build_nc(repeats=1, exchange=True):
    """repeats>1 re-emits the whole computation R times back-to-back in one
    NEFF — used by the bench harness to measure device time differentially
    (dispatch overhead cancels in t(R) - t(1)).

    exchange=False falls back to computing full K/V on every core (no
    collectives, ~20% more PE work)."""
    nc = bacc.Bacc("TRN2", target_bir_lowering=False, debug=False, num_devices=N_CORES)

    xT = nc.declare_dram_parameter("xT", [C, T], BF16, isOutput=False)
    W = nc.declare_dram_parameter("W", [C, 3 * C], BF16, isOutput=False)
    bq = nc.declare_dram_parameter("bq", [128, CT], F32, isOutput=False)
    bk = nc.declare_dram_parameter("bk", [128, CT], F32, isOutput=False)
    bvr = nc.declare_dram_parameter("bvr", [128, C], F32, isOutput=False)
    out = nc.declare_dram_parameter("out", [TQ, C], F32, isOutput=True)

    with ExitStack() as ctx:
        tc = ctx.enter_context(tile.TileContext(nc))
        singles = ctx.enter_context(tc.tile_pool(name="singles", bufs=1))
        wpool = ctx.enter_context(tc.tile_pool(name="wpool", bufs=2))
        opool = ctx.enter_context(tc.tile_pool(name="opool", bufs=2))
        stage = ctx.enter_context(tc.tile_pool(name="stage", bufs=4))
        small = ctx.enter_context(tc.tile_pool(name="small", bufs=2))
        psum = ctx.enter_context(tc.tile_pool(name="psum", bufs=3, space="PSUM"))
        dram = ctx.enter_context(tc.tile_pool(name="dram", bufs=1, space="DRAM"))

        # ---- resident inputs -------------------------------------------------
        xT_sb = singles.tile([128, CT, T], BF16)  # [p, j, t] = xT[j*128+p, t]
        for j in range(CT):
            nc.sync.dma_start(out=xT_sb[:, j, :], in_=xT[j * 128 : (j + 1) * 128, :])
        bq_sb = singles.tile([128, CT], F32)
        nc.sync.dma_start(out=bq_sb, in_=bq[:, :])
        bk_sb = singles.tile([128, CT], F32)
        nc.sync.dma_start(out=bk_sb, in_=bk[:, :])
        bvr_sb = singles.tile([128, C], F32)
        nc.sync.dma_start(out=bvr_sb, in_=bvr[:, :])
        ones_sb = singles.tile([128, 1], BF16)
        nc.vector.memset(ones_sb, 1.0)

        QT_sb = singles.tile([128, CT, TQ], BF16)
        KT_sb = singles.tile([128, CT, T], BF16)
        V_sb = singles.tile([128, ST, C], BF16)
        attT_sb = singles.tile([128, ST, TQ], BF16)

        def load_w_third(col0):
            w_sb = wpool.tile([128, CT, C], BF16, tag="w")
            for j in range(CT):
                nc.sync.dma_start(
                    out=w_sb[:, j, :],
                    in_=W[j * 128 : (j + 1) * 128, col0 : col0 + C],
                )
            return w_sb

        def mm_proj_T(ps, w_sb, m, lo, width):
            """psum[c-tile m rows, t in [lo, lo+width)] = W-strip.T @ xT."""
            for j in range(CT):
                nc.tensor.matmul(
                    ps,
                    w_sb[:, j, m * 128 : (m + 1) * 128],
                    xT_sb[:, j, lo : lo + width],
                    start=(j == 0),
                    stop=(j == CT - 1),
                )

        def emit_k_proj(wk_sb, s_cols):
            """KT for s in [0, s_cols); returns via staging to kt_send DRAM
            (exchange) or directly into KT_sb (no exchange)."""
            kt_send = None
            if exchange:
                kt_send = dram.tile([CT, 128, s_cols], BF16, tag="kt_send")
            for m in range(CT):
                for s2 in range(s_cols // 512):
                    ps = psum.tile([128, 512], F32, tag="mm")
                    mm_proj_T(ps, wk_sb, m, s2 * 512, 512)
                    if exchange:
                        st_sb = stage.tile([128, 512], BF16, tag="stage")
                        nc.vector.tensor_scalar_add(st_sb, ps, bk_sb[:, m : m + 1])
                        nc.sync.dma_start(
                            out=kt_send[m, :, s2 * 512 : (s2 + 1) * 512], in_=st_sb
                        )
                    else:
                        nc.vector.tensor_scalar_add(
                            KT_sb[:, m, s2 * 512 : (s2 + 1) * 512],
                            ps,
                            bk_sb[:, m : m + 1],
                        )
            return kt_send

        def emit_q_proj(wq_sb):
            for m in range(CT):
                for q2 in range(TQ // 512):
                    ps = psum.tile([128, 512], F32, tag="mm")
                    mm_proj_T(ps, wq_sb, m, q2 * 512, 512)
                    nc.vector.tensor_scalar_add(
                        QT_sb[:, m, q2 * 512 : (q2 + 1) * 512], ps, bq_sb[:, m : m + 1]
                    )

        def emit_v_proj(wv_sb, n_stiles):
            """V for s-tiles [0, n_stiles); via staging to v_send (exchange)
            or directly into V_sb (no exchange)."""
            v_send = None
            if exchange:
                v_send = dram.tile([STH, 128, C], BF16, tag="v_send")
            for st in range(n_stiles):
                for c2 in range(C // 512):
                    ps = psum.tile([128, 512], F32, tag="mm")
                    for j in range(CT):
                        nc.tensor.matmul(
                            ps,
                            xT_sb[:, j, st * 128 : (st + 1) * 128],
                            wv_sb[:, j, c2 * 512 : (c2 + 1) * 512],
                            start=(j == 0),
                            stop=(j == CT - 1),
                        )
                    if exchange:
                        st_sb = stage.tile([128, 512], BF16, tag="stage")
                        nc.scalar.activation(
                            out=st_sb, in_=ps, func=mybir.ActivationFunctionType.Copy
                        )
                        nc.sync.dma_start(
                            out=v_send[st, :, c2 * 512 : (c2 + 1) * 512], in_=st_sb
                        )
                    else:
                        nc.scalar.activation(
                            out=V_sb[:, st, c2 * 512 : (c2 + 1) * 512],
                            in_=ps,
                            func=mybir.ActivationFunctionType.Copy,
                        )
            return v_send

        def emit_scores():
            # scores (transposed) + exp: attT[s, tq] = exp(KT.T-strips @ QT / 32)
            for q2 in range(TQ // 512):
                for st in range(ST):
                    ps = psum.tile([128, 512], F32, tag="mm")
                    for m in range(CT):
                        nc.tensor.matmul(
                            ps,
                            KT_sb[:, m, st * 128 : (st + 1) * 128],
                            QT_sb[:, m, q2 * 512 : (q2 + 1) * 512],
                            start=(m == 0),
                            stop=(m == CT - 1),
                        )
                    nc.scalar.activation(
                        out=attT_sb[:, st, q2 * 512 : (q2 + 1) * 512],
                        in_=ps,
                        func=mybir.ActivationFunctionType.Exp,
                        scale=1.0 / 32.0,
                    )

        def emit_av():
            # AV + denominators + normalize + v-bias + store
            for i in range(TQ // 128):
                av = psum.tile([128, C], F32, tag="av", bufs=2)
                sS = psum.tile([128, 1], F32, tag="s", bufs=1)
                for st in range(ST):
                    lhsT = attT_sb[:, st, i * 128 : (i + 1) * 128]
                    nc.tensor.matmul(
                        av[:, 0:512],
                        lhsT,
                        V_sb[:, st, 0:512],
                        start=(st == 0),
                        stop=(st == ST - 1),
                    )
                    nc.tensor.matmul(
                        av[:, 512:1024],
                        lhsT,
                        V_sb[:, st, 512:1024],
                        start=(st == 0),
                        stop=(st == ST - 1),
                    )
                    nc.tensor.matmul(
                        sS, lhsT, ones_sb, start=(st == 0), stop=(st == ST - 1)
                    )
                recip = small.tile([128, 1], F32, tag="recip")
                nc.vector.reciprocal(recip, sS)
                o_sb = opool.tile([128, C], F32, tag="o")
                nc.vector.tensor_scalar_mul(o_sb[:, 0:512], av[:, 0:512], recip)
                nc.vector.tensor_scalar_mul(o_sb[:, 512:1024], av[:, 512:1024], recip)
                nc.vector.tensor_add(o_sb, o_sb, bvr_sb)
                nc.sync.dma_start(out=out[i * 128 : (i + 1) * 128, :], in_=o_sb)

        for _ in range(repeats):
            if exchange:
                # K first so its exchange overlaps the Q/V projections.
                wk_sb = load_w_third(C)
                kt_send = emit_k_proj(wk_sb, TQ)
                kt_recv = dram.tile([2, CT, 128, TQ], BF16, tag="kt_recv")
                nc.gpsimd.collective_compute(
                    "AllGather",
                    mybir.AluOpType.bypass,
                    replica_groups=PAIRS,
                    ins=[kt_send.opt()],
                    outs=[kt_recv.opt()],
                )
                for r in range(2):
                    for m in range(CT):
                        nc.sync.dma_start(
                            out=KT_sb[:, m, r * TQ : (r + 1) * TQ],
                            in_=kt_recv[r, m, :, :],
                        )

                wq_sb = load_w_third(0)
                emit_q_proj(wq_sb)

                wv_sb = load_w_third(2 * C)
                v_send = emit_v_proj(wv_sb, STH)
                v_recv = dram.tile([2, STH, 128, C], BF16, tag="v_recv")
                nc.gpsimd.collective_compute(
                    "AllGather",
                    mybir.AluOpType.bypass,
                    replica_groups=PAIRS,
                    ins=[v_send.opt()],
                    outs=[v_recv.opt()],
                )
                for r in range(2):
                    for st in range(STH):
                        nc.sync.dma_start(
                            out=V_sb[:, r * STH + st, :], in_=v_recv[r, st, :, :]
                        )
            else:
                wk_sb = load_w_third(C)
                emit_k_proj(wk_sb, T)
                wq_sb = load_w_third(0)
                emit_q_proj(wq_sb)
                wv_sb = load_w_third(2 * C)
                emit_v_proj(wv_sb, ST)

            emit_scores()
            emit_av()

    nc.compile()
    return nc


def _get_nc():
    global _NC_CACHE
    if _NC_CACHE is None:
        _NC_CACHE = _build_nc()
    return _NC_CACHE


def _make_in_maps(x, W_qkv, b_qkv):
    W_b16 = np.ascontiguousarray(W_qkv.astype(ml_dtypes.bfloat16))
    # biases laid out [128, CT]: col j holds bias[j*128 : (j+1)*128]
    bq = np.ascontiguousarray(b_qkv[:C].reshape(CT, 128).T.astype(np.float32))
    bk = np.ascontiguousarray(b_qkv[C : 2 * C].reshape(CT, 128).T.astype(np.float32))
    bvr = np.ascontiguousarray(
        np.broadcast_to(b_qkv[2 * C :].astype(np.float32), (128, C))
    )
    in_maps = []
    for core in range(N_CORES):
        b, h = core // 2, core % 2
        xb = x[b]
        if h == 1:  # roll so this core's query half comes first
            xb = np.concatenate([xb[TQ:], xb[:TQ]], axis=0)
        xT = np.ascontiguousarray(xb.T).astype(ml_dtypes.bfloat16)
        in_maps.append({"xT": xT, "W": W_b16, "bq": bq, "bk": bk, "bvr": bvr})
    return in_maps


def _run(x, W_qkv, b_qkv, trace=False, **spmd_kwargs):
    nc = _get_nc()
    in_maps = _make_in_maps(x, W_qkv, b_qkv)
    res = run_bass_kernel_spmd(
        nc, in_maps, list(range(N_CORES)), trace=trace, **spmd_kwargs
    )
    out = np.empty((B, T, C), dtype=np.float32)
    for core in range(N_CORES):
        b, h = core // 2, core % 2
        out[b, h * TQ : (h + 1) * TQ, :] = res.results[core]["out"]
    return out, res


def kernel(x, W_qkv, b_qkv):
    x = np.asarray(x)
    W_qkv = np.asarray(W_qkv)
    b_qkv = np.asarray(b_qkv)
    out, _ = _run(x, W_qkv, b_qkv)
    return out


# revision 14
# speedup vs baseline: 1.5127x; 1.3309x over previous
"""Trainium2 Bass kernel for fused single-head attention.

Reference (jax, fp32):
    qkv = x @ W_qkv + b_qkv            # [B,T,C] @ [C,3C]
    q, k, v = split(qkv, 3, axis=-1)
    att = softmax(q @ k.T / sqrt(C))   # per batch, [T,T]
    out = att @ v                      # [B,T,C]

Shapes: B=4, T=2048, C=1024, fp32 in/out.

Sharding (8 cores, pure SPMD): core c handles batch b = c//2 and
query-row half h = c%2 (1024 query rows). Each core projects K and V
for only its own 1024 rows; the halves are exchanged between the two
cores of a batch with a pairwise AllGather (groups [0,1],[2,3],...),
so no projection work is duplicated. The K exchange is issued right
after the K projection so it overlaps the Q and V projections on the
PE; the V exchange overlaps the score matmuls.

Per-core layout trick: the kernel program must be identical on all
cores (SPMD), so instead of slicing a different query-half per core,
the host rolls the T axis of x so that the core's own 1024 query rows
always come first. Attention is invariant to a permutation of the
key/value axis; after the exchange both cores of a pair use the same
pair-global order [even-core half, odd-core half] for K/V/att.

On-device dataflow (all matmuls bf16 operands, fp32 PSUM accumulate):
  xT  [C, T]   (transposed by host, bf16)
  KT-own [c, 1024] = Wk.T-stationary @ xT[:, :1024]  (+bk)  → AllGather
  QT  [c, tq] = Wq.T-stationary @ xT[:, :1024]       (+bq, kept transposed)
  V-own  [1024, c] = xT-stationary @ Wv              (no bias) → AllGather
  SCT [s, tq] = KT-stationary @ QT        (scores, transposed)
  attT[s, tq] = exp(SCT / 32)             (no max-subtraction: logits are
                                           O(3) by construction, exp is safe)
  U   [tq, c] = attT-stationary @ V       (unnormalized output)
  S   [tq, 1] = attT-stationary @ ones    (softmax denominators)
  out = U * (1/S) + bv                    (softmax normalization + v-bias:
                                           att/S rows sum to 1 exactly, so
                                           adding bv at the end is exact)
"""

from contextlib import ExitStack

import ml_dtypes
import numpy as np

import concourse.tile as tile
from concourse import bacc, mybir
from concourse.bass_utils import run_bass_kernel_spmd

B, T, C = 4, 2048, 1024
N_CORES = 8
TQ = T // 2          # query rows per core
CT = C // 128        # 8 c-tiles
ST = T // 128        # 16 s-tiles
STH = ST // 2        # own-half s-tiles
BF16 = mybir.dt.bfloat16
F32 = mybir.dt.float32
PAIRS = [[0, 1], [2, 3], [4, 5], [6, 7]]

_NC_CACHE = None


def _build_nc(repeats=1, exchange=True, no_cc=False, phases=("proj", "scores", "av")):
    """repeats>1 re-emits the whole computation R times back-to-back in one
    NEFF — used by the bench harness to measure device time differentially
    (dispatch overhead cancels in t(R) - t(1)).

    exchange=False falls back to computing full K/V on every core (no
    collectives, ~20% more PE work). no_cc=True keeps the exchange program
    shape but skips the collectives (timing ablation only — wrong data)."""
    nc = bacc.Bacc("TRN2", target_bir_lowering=False, debug=False, num_devices=N_CORES)

    xT = nc.declare_dram_parameter("xT", [C, T], BF16, isOutput=False)
    W = nc.declare_dram_parameter("W", [C, 3 * C], BF16, isOutput=False)
    bq = nc.declare_dram_parameter("bq", [128, CT], F32, isOutput=False)
    bk = nc.declare_dram_parameter("bk", [128, CT], F32, isOutput=False)
    bvr = nc.declare_dram_parameter("bvr", [128, C], F32, isOutput=False)
    out = nc.declare_dram_parameter("out", [TQ, C], F32, isOutput=True)

    with ExitStack() as ctx:
        tc = ctx.enter_context(tile.TileContext(nc))
        singles = ctx.enter_context(tc.tile_pool(name="singles", bufs=1))
        wpool = ctx.enter_context(tc.tile_pool(name="wpool", bufs=2))
        opool = ctx.enter_context(tc.tile_pool(name="opool", bufs=2))
        stage = ctx.enter_context(tc.tile_pool(name="stage", bufs=4))
        small = ctx.enter_context(tc.tile_pool(name="small", bufs=2))
        psum = ctx.enter_context(tc.tile_pool(name="psum", bufs=3, space="PSUM"))
        dram = ctx.enter_context(tc.tile_pool(name="dram", bufs=1, space="DRAM"))

        # ---- resident inputs -------------------------------------------------
        xT_sb = singles.tile([128, CT, T], BF16)  # [p, j, t] = xT[j*128+p, t]
        for j in range(CT):
            nc.sync.dma_start(out=xT_sb[:, j, :], in_=xT[j * 128 : (j + 1) * 128, :])
        bq_sb = singles.tile([128, CT], F32)
        nc.sync.dma_start(out=bq_sb, in_=bq[:, :])
        bk_sb = singles.tile([128, CT], F32)
        nc.sync.dma_start(out=bk_sb, in_=bk[:, :])
        bvr_sb = singles.tile([128, C], F32)
        nc.sync.dma_start(out=bvr_sb, in_=bvr[:, :])
        ones_sb = singles.tile([128, 1], BF16)
        nc.vector.memset(ones_sb, 1.0)

        QT_sb = singles.tile([128, CT, TQ], BF16)
        KT_sb = singles.tile([128, CT, T], BF16)
        V_sb = singles.tile([128, ST, C], BF16)
        attT_sb = singles.tile([128, ST, TQ], BF16)

        def load_w_third(col0):
            w_sb = wpool.tile([128, CT, C], BF16, tag="w")
            for j in range(CT):
                nc.sync.dma_start(
                    out=w_sb[:, j, :],
                    in_=W[j * 128 : (j + 1) * 128, col0 : col0 + C],
                )
            return w_sb

        def mm_proj_T(ps, w_sb, m, lo, width):
            """psum[c-tile m rows, t in [lo, lo+width)] = W-strip.T @ xT."""
            for j in range(CT):
                nc.tensor.matmul(
                    ps,
                    w_sb[:, j, m * 128 : (m + 1) * 128],
                    xT_sb[:, j, lo : lo + width],
                    start=(j == 0),
                    stop=(j == CT - 1),
                )

        def emit_k_proj_exchange(wk_sb):
            """KT for own s in [0, TQ), split into 512-col chunks, each chunk
            exchanged with its own pairwise AllGather so the first collective
            launches ~14us into the iteration and both are hidden behind the
            Q/V projections. Readback lands both ranks' halves into KT_sb in
            pair-global order."""
            for s2 in range(TQ // 512):
                kt_send = dram.tile([CT, 128, 512], BF16, tag=f"kt_send{s2}")
                for m in range(CT):
                    ps = psum.tile([128, 512], F32, tag="mm")
                    mm_proj_T(ps, wk_sb, m, s2 * 512, 512)
                    st_sb = stage.tile([128, 512], BF16, tag="stage")
                    nc.vector.tensor_scalar_add(st_sb, ps, bk_sb[:, m : m + 1])
                    nc.sync.dma_start(out=kt_send[m, :, :], in_=st_sb)
                kt_recv = dram.tile([2, CT, 128, 512], BF16, tag=f"kt_recv{s2}")
                if not no_cc:
                    nc.gpsimd.collective_compute(
                        "AllGather",
                        mybir.AluOpType.bypass,
                        replica_groups=PAIRS,
                        ins=[kt_send.opt()],
                        outs=[kt_recv.opt()],
                    )
                for r in range(2):
                    for m in range(CT):
                        nc.sync.dma_start(
                            out=KT_sb[:, m, r * TQ + s2 * 512 : r * TQ + (s2 + 1) * 512],
                            in_=kt_recv[r, m, :, :],
                        )

        def emit_k_proj_local(wk_sb):
            """KT for all of s directly into KT_sb (no exchange)."""
            for m in range(CT):
                for s2 in range(T // 512):
                    ps = psum.tile([128, 512], F32, tag="mm")
                    mm_proj_T(ps, wk_sb, m, s2 * 512, 512)
                    nc.vector.tensor_scalar_add(
                        KT_sb[:, m, s2 * 512 : (s2 + 1) * 512],
                        ps,
                        bk_sb[:, m : m + 1],
                    )

        def emit_q_proj(wq_sb):
            for m in range(CT):
                for q2 in range(TQ // 512):
                    ps = psum.tile([128, 512], F32, tag="mm")
                    mm_proj_T(ps, wq_sb, m, q2 * 512, 512)
                    nc.vector.tensor_scalar_add(
                        QT_sb[:, m, q2 * 512 : (q2 + 1) * 512], ps, bq_sb[:, m : m + 1]
                    )

        def emit_v_proj(wv_sb, n_stiles):
            """V for s-tiles [0, n_stiles); via staging to v_send (exchange)
            or directly into V_sb (no exchange)."""
            v_send = None
            if exchange:
                v_send = dram.tile([STH, 128, C], BF16, tag="v_send")
            for st in range(n_stiles):
                for c2 in range(C // 512):
                    ps = psum.tile([128, 512], F32, tag="mm")
                    for j in range(CT):
                        nc.tensor.matmul(
                            ps,
                            xT_sb[:, j, st * 128 : (st + 1) * 128],
                            wv_sb[:, j, c2 * 512 : (c2 + 1) * 512],
                            start=(j == 0),
                            stop=(j == CT - 1),
                        )
                    if exchange:
                        st_sb = stage.tile([128, 512], BF16, tag="stage")
                        nc.scalar.activation(
                            out=st_sb, in_=ps, func=mybir.ActivationFunctionType.Copy
                        )
                        nc.sync.dma_start(
                            out=v_send[st, :, c2 * 512 : (c2 + 1) * 512], in_=st_sb
                        )
                    else:
                        nc.scalar.activation(
                            out=V_sb[:, st, c2 * 512 : (c2 + 1) * 512],
                            in_=ps,
                            func=mybir.ActivationFunctionType.Copy,
                        )
            return v_send

        def emit_scores():
            # scores (transposed) + exp: attT[s, tq] = exp(KT.T-strips @ QT / 32)
            for q2 in range(TQ // 512):
                for st in range(ST):
                    ps = psum.tile([128, 512], F32, tag="mm")
                    for m in range(CT):
                        nc.tensor.matmul(
                            ps,
                            KT_sb[:, m, st * 128 : (st + 1) * 128],
                            QT_sb[:, m, q2 * 512 : (q2 + 1) * 512],
                            start=(m == 0),
                            stop=(m == CT - 1),
                        )
                    nc.scalar.activation(
                        out=attT_sb[:, st, q2 * 512 : (q2 + 1) * 512],
                        in_=ps,
                        func=mybir.ActivationFunctionType.Exp,
                        scale=1.0 / 32.0,
                    )

        def emit_av():
            # AV + denominators + normalize + v-bias + store
            for i in range(TQ // 128):
                av = psum.tile([128, C], F32, tag="av", bufs=2)
                sS = psum.tile([128, 1], F32, tag="s", bufs=1)
                for st in range(ST):
                    lhsT = attT_sb[:, st, i * 128 : (i + 1) * 128]
                    nc.tensor.matmul(
                        av[:, 0:512],
                        lhsT,
                        V_sb[:, st, 0:512],
                        start=(st == 0),
                        stop=(st == ST - 1),
                    )
                    nc.tensor.matmul(
                        av[:, 512:1024],
                        lhsT,
                        V_sb[:, st, 512:1024],
                        start=(st == 0),
                        stop=(st == ST - 1),
                    )
                    nc.tensor.matmul(
                        sS, lhsT, ones_sb, start=(st == 0), stop=(st == ST - 1)
                    )
                recip = small.tile([128, 1], F32, tag="recip")
                nc.vector.reciprocal(recip, sS)
                o_sb = opool.tile([128, C], F32, tag="o")
                nc.vector.tensor_scalar_mul(o_sb[:, 0:512], av[:, 0:512], recip)
                nc.vector.tensor_scalar_mul(o_sb[:, 512:1024], av[:, 512:1024], recip)
                nc.vector.tensor_add(o_sb, o_sb, bvr_sb)
                nc.sync.dma_start(out=out[i * 128 : (i + 1) * 128, :], in_=o_sb)

        for _ in range(repeats):
            if "proj" in phases:
                if exchange:
                    # K first so its exchange overlaps the Q/V projections.
                    wk_sb = load_w_third(C)
                    emit_k_proj_exchange(wk_sb)

                    wq_sb = load_w_third(0)
                    emit_q_proj(wq_sb)

                    wv_sb = load_w_third(2 * C)
                    v_send = emit_v_proj(wv_sb, STH)
                    v_recv = dram.tile([2, STH, 128, C], BF16, tag="v_recv")
                    if not no_cc:
                        nc.gpsimd.collective_compute(
                            "AllGather",
                            mybir.AluOpType.bypass,
                            replica_groups=PAIRS,
                            ins=[v_send.opt()],
                            outs=[v_recv.opt()],
                        )
                    for r in range(2):
                        for st in range(STH):
                            nc.sync.dma_start(
                                out=V_sb[:, r * STH + st, :], in_=v_recv[r, st, :, :]
                            )
                else:
                    wk_sb = load_w_third(C)
                    emit_k_proj_local(wk_sb)
                    wq_sb = load_w_third(0)
                    emit_q_proj(wq_sb)
                    wv_sb = load_w_third(2 * C)
                    emit_v_proj(wv_sb, ST)

            if "scores" in phases:
                emit_scores()
            if "av" in phases:
                emit_av()

    nc.compile()
    return nc


def _get_nc():
    global _NC_CACHE
    if _NC_CACHE is None:
        _NC_CACHE = _build_nc()
    return _NC_CACHE


def _make_in_maps(x, W_qkv, b_qkv):
    W_b16 = np.ascontiguousarray(W_qkv.astype(ml_dtypes.bfloat16))
    # biases laid out [128, CT]: col j holds bias[j*128 : (j+1)*128]
    bq = np.ascontiguousarray(b_qkv[:C].reshape(CT, 128).T.astype(np.float32))
    bk = np.ascontiguousarray(b_qkv[C : 2 * C].reshape(CT, 128).T.astype(np.float32))
    bvr = np.ascontiguousarray(
        np.broadcast_to(b_qkv[2 * C :].astype(np.float32), (128, C))
    )
    in_maps = []
    for core in range(N_CORES):
        b, h = core // 2, core % 2
        xb = x[b]
        if h == 1:  # roll so this core's query half comes first
            xb = np.concatenate([xb[TQ:], xb[:TQ]], axis=0)
        xT = np.ascontiguousarray(xb.T).astype(ml_dtypes.bfloat16)
        in_maps.append({"xT": xT, "W": W_b16, "bq": bq, "bk": bk, "bvr": bvr})
    return in_maps


def _run(x, W_qkv, b_qkv, trace=False, **spmd_kwargs):
    nc = _get_nc()
    in_maps = _make_in_maps(x, W_qkv, b_qkv)
    res = run_bass_kernel_spmd(
        nc, in_maps, list(range(N_CORES)), trace=trace, **spmd_kwargs
    )
    out = np.empty((B, T, C), dtype=np.float32)
    for core in range(N_CORES):
        b, h = core // 2, core % 2
        out[b, h * TQ : (h + 1) * TQ, :] = res.results[core]["out"]
    return out, res


def kernel(x, W_qkv, b_qkv):
    x = np.asarray(x)
    W_qkv = np.asarray(W_qkv)
    b_qkv = np.asarray(b_qkv)
    out, _ = _run(x, W_qkv, b_qkv)
    return out


# revision 17
# speedup vs baseline: 1.5318x; 1.0126x over previous
"""Trainium2 Bass kernel for fused single-head attention.

Reference (jax, fp32):
    qkv = x @ W_qkv + b_qkv            # [B,T,C] @ [C,3C]
    q, k, v = split(qkv, 3, axis=-1)
    att = softmax(q @ k.T / sqrt(C))   # per batch, [T,T]
    out = att @ v                      # [B,T,C]

Shapes: B=4, T=2048, C=1024, fp32 in/out.

Sharding (8 cores, pure SPMD): core c handles batch b = c//2 and
query-row half h = c%2 (1024 query rows). Each core projects K and V
for only its own 1024 rows; the halves are exchanged between the two
cores of a batch with a pairwise AllGather (groups [0,1],[2,3],...),
so no projection work is duplicated. The K exchange is issued right
after the K projection so it overlaps the Q and V projections on the
PE; the V exchange overlaps the score matmuls.

Per-core layout trick: the kernel program must be identical on all
cores (SPMD), so instead of slicing a different query-half per core,
the host rolls the T axis of x so that the core's own 1024 query rows
always come first. Attention is invariant to a permutation of the
key/value axis; after the exchange both cores of a pair use the same
pair-global order [even-core half, odd-core half] for K/V/att.

On-device dataflow (all matmuls bf16 operands, fp32 PSUM accumulate):
  xT  [C, T]   (transposed by host, bf16)
  KT-own [c, 1024] = Wk.T-stationary @ xT[:, :1024]  (+bk)  → AllGather
  QT  [c, tq] = Wq.T-stationary @ xT[:, :1024]       (+bq, kept transposed)
  V-own  [1024, c] = xT-stationary @ Wv              (no bias) → AllGather
  SCT [s, tq] = KT-stationary @ QT        (scores, transposed)
  attT[s, tq] = exp(SCT / 32)             (no max-subtraction: logits are
                                           O(3) by construction, exp is safe)
  U   [tq, c] = attT-stationary @ V       (unnormalized output)
  S   [tq, 1] = attT-stationary @ ones    (softmax denominators)
  out = U * (1/S) + bv                    (softmax normalization + v-bias:
                                           att/S rows sum to 1 exactly, so
                                           adding bv at the end is exact)
"""

from contextlib import ExitStack

import ml_dtypes
import numpy as np

import concourse.tile as tile
from concourse import bacc, mybir
from concourse.bass_utils import run_bass_kernel_spmd

B, T, C = 4, 2048, 1024
N_CORES = 8
TQ = T // 2          # query rows per core
CT = C // 128        # 8 c-tiles
ST = T // 128        # 16 s-tiles
STH = ST // 2        # own-half s-tiles
BF16 = mybir.dt.bfloat16
F32 = mybir.dt.float32
PAIRS = [[0, 1], [2, 3], [4, 5], [6, 7]]

_NC_CACHE = None


def _build_nc(
    repeats=1,
    exchange=True,
    no_cc=False,
    phases=("proj", "scores", "av"),
    fuse_out=True,
    stage_bufs=8,
):
    """repeats>1 re-emits the whole computation R times back-to-back in one
    NEFF — used by the bench harness to measure device time differentially
    (dispatch overhead cancels in t(R) - t(1)).

    exchange=False falls back to computing full K/V on every core (no
    collectives, ~20% more PE work). no_cc=True keeps the exchange program
    shape but skips the collectives (timing ablation only — wrong data)."""
    nc = bacc.Bacc("TRN2", target_bir_lowering=False, debug=False, num_devices=N_CORES)

    xT = nc.declare_dram_parameter("xT", [C, T], BF16, isOutput=False)
    W = nc.declare_dram_parameter("W", [C, 3 * C], BF16, isOutput=False)
    bq = nc.declare_dram_parameter("bq", [128, CT], F32, isOutput=False)
    bk = nc.declare_dram_parameter("bk", [128, CT], F32, isOutput=False)
    bvr = nc.declare_dram_parameter("bvr", [128, C], F32, isOutput=False)
    out = nc.declare_dram_parameter("out", [TQ, C], F32, isOutput=True)

    with ExitStack() as ctx:
        tc = ctx.enter_context(tile.TileContext(nc))
        singles = ctx.enter_context(tc.tile_pool(name="singles", bufs=1))
        wpool = ctx.enter_context(tc.tile_pool(name="wpool", bufs=2))
        opool = ctx.enter_context(tc.tile_pool(name="opool", bufs=2))
        stage = ctx.enter_context(tc.tile_pool(name="stage", bufs=stage_bufs))
        small = ctx.enter_context(tc.tile_pool(name="small", bufs=2))
        psum = ctx.enter_context(tc.tile_pool(name="psum", bufs=3, space="PSUM"))
        dram = ctx.enter_context(tc.tile_pool(name="dram", bufs=1, space="DRAM"))

        # ---- resident inputs -------------------------------------------------
        xT_sb = singles.tile([128, CT, T], BF16)  # [p, j, t] = xT[j*128+p, t]
        for j in range(CT):
            nc.sync.dma_start(out=xT_sb[:, j, :], in_=xT[j * 128 : (j + 1) * 128, :])
        bq_sb = singles.tile([128, CT], F32)
        nc.sync.dma_start(out=bq_sb, in_=bq[:, :])
        bk_sb = singles.tile([128, CT], F32)
        nc.sync.dma_start(out=bk_sb, in_=bk[:, :])
        bvr_sb = singles.tile([128, C], F32)
        nc.sync.dma_start(out=bvr_sb, in_=bvr[:, :])
        ones_sb = singles.tile([128, 1], BF16)
        nc.vector.memset(ones_sb, 1.0)

        QT_sb = singles.tile([128, CT, TQ], BF16)
        KT_sb = singles.tile([128, CT, T], BF16)
        V_sb = singles.tile([128, ST, C], BF16)
        attT_sb = singles.tile([128, ST, TQ], BF16)

        def load_w_third(col0):
            w_sb = wpool.tile([128, CT, C], BF16, tag="w")
            for j in range(CT):
                nc.sync.dma_start(
                    out=w_sb[:, j, :],
                    in_=W[j * 128 : (j + 1) * 128, col0 : col0 + C],
                )
            return w_sb

        def mm_proj_T(ps, w_sb, m, lo, width):
            """psum[c-tile m rows, t in [lo, lo+width)] = W-strip.T @ xT."""
            for j in range(CT):
                nc.tensor.matmul(
                    ps,
                    w_sb[:, j, m * 128 : (m + 1) * 128],
                    xT_sb[:, j, lo : lo + width],
                    start=(j == 0),
                    stop=(j == CT - 1),
                )

        def emit_k_proj_exchange(wk_sb):
            """KT for own s in [0, TQ), split into 512-col chunks, each chunk
            exchanged with its own pairwise AllGather so the first collective
            launches ~14us into the iteration and both are hidden behind the
            Q/V projections. Readback lands both ranks' halves into KT_sb in
            pair-global order."""
            for s2 in range(TQ // 512):
                kt_send = dram.tile([CT, 128, 512], BF16, tag=f"kt_send{s2}")
                for m in range(CT):
                    ps = psum.tile([128, 512], F32, tag="mm")
                    mm_proj_T(ps, wk_sb, m, s2 * 512, 512)
                    st_sb = stage.tile([128, 512], BF16, tag="stage")
                    nc.vector.tensor_scalar_add(st_sb, ps, bk_sb[:, m : m + 1])
                    nc.sync.dma_start(out=kt_send[m, :, :], in_=st_sb)
                kt_recv = dram.tile([2, CT, 128, 512], BF16, tag=f"kt_recv{s2}")
                if not no_cc:
                    nc.gpsimd.collective_compute(
                        "AllGather",
                        mybir.AluOpType.bypass,
                        replica_groups=PAIRS,
                        ins=[kt_send.opt()],
                        outs=[kt_recv.opt()],
                    )
                for r in range(2):
                    for m in range(CT):
                        nc.sync.dma_start(
                            out=KT_sb[:, m, r * TQ + s2 * 512 : r * TQ + (s2 + 1) * 512],
                            in_=kt_recv[r, m, :, :],
                        )

        def emit_k_proj_local(wk_sb):
            """KT for all of s directly into KT_sb (no exchange)."""
            for m in range(CT):
                for s2 in range(T // 512):
                    ps = psum.tile([128, 512], F32, tag="mm")
                    mm_proj_T(ps, wk_sb, m, s2 * 512, 512)
                    nc.vector.tensor_scalar_add(
                        KT_sb[:, m, s2 * 512 : (s2 + 1) * 512],
                        ps,
                        bk_sb[:, m : m + 1],
                    )

        def emit_q_proj(wq_sb):
            for m in range(CT):
                for q2 in range(TQ // 512):
                    ps = psum.tile([128, 512], F32, tag="mm")
                    mm_proj_T(ps, wq_sb, m, q2 * 512, 512)
                    nc.vector.tensor_scalar_add(
                        QT_sb[:, m, q2 * 512 : (q2 + 1) * 512], ps, bq_sb[:, m : m + 1]
                    )

        def emit_v_proj(wv_sb, n_stiles):
            """V for s-tiles [0, n_stiles); via staging to v_send (exchange)
            or directly into V_sb (no exchange)."""
            v_send = None
            if exchange:
                v_send = dram.tile([STH, 128, C], BF16, tag="v_send")
            for st in range(n_stiles):
                for c2 in range(C // 512):
                    ps = psum.tile([128, 512], F32, tag="mm")
                    for j in range(CT):
                        nc.tensor.matmul(
                            ps,
                            xT_sb[:, j, st * 128 : (st + 1) * 128],
                            wv_sb[:, j, c2 * 512 : (c2 + 1) * 512],
                            start=(j == 0),
                            stop=(j == CT - 1),
                        )
                    if exchange:
                        st_sb = stage.tile([128, 512], BF16, tag="stage")
                        nc.scalar.activation(
                            out=st_sb, in_=ps, func=mybir.ActivationFunctionType.Copy
                        )
                        nc.sync.dma_start(
                            out=v_send[st, :, c2 * 512 : (c2 + 1) * 512], in_=st_sb
                        )
                    else:
                        nc.scalar.activation(
                            out=V_sb[:, st, c2 * 512 : (c2 + 1) * 512],
                            in_=ps,
                            func=mybir.ActivationFunctionType.Copy,
                        )
            return v_send

        def emit_scores():
            # scores (transposed) + exp: attT[s, tq] = exp(KT.T-strips @ QT / 32)
            for q2 in range(TQ // 512):
                for st in range(ST):
                    ps = psum.tile([128, 512], F32, tag="mm")
                    for m in range(CT):
                        nc.tensor.matmul(
                            ps,
                            KT_sb[:, m, st * 128 : (st + 1) * 128],
                            QT_sb[:, m, q2 * 512 : (q2 + 1) * 512],
                            start=(m == 0),
                            stop=(m == CT - 1),
                        )
                    nc.scalar.activation(
                        out=attT_sb[:, st, q2 * 512 : (q2 + 1) * 512],
                        in_=ps,
                        func=mybir.ActivationFunctionType.Exp,
                        scale=1.0 / 32.0,
                    )

        def emit_av():
            # AV + denominators + normalize + v-bias + store
            for i in range(TQ // 128):
                av = psum.tile([128, C], F32, tag="av", bufs=2)
                sS = psum.tile([128, 1], F32, tag="s", bufs=1)
                for st in range(ST):
                    lhsT = attT_sb[:, st, i * 128 : (i + 1) * 128]
                    nc.tensor.matmul(
                        av[:, 0:512],
                        lhsT,
                        V_sb[:, st, 0:512],
                        start=(st == 0),
                        stop=(st == ST - 1),
                    )
                    nc.tensor.matmul(
                        av[:, 512:1024],
                        lhsT,
                        V_sb[:, st, 512:1024],
                        start=(st == 0),
                        stop=(st == ST - 1),
                    )
                    nc.tensor.matmul(
                        sS, lhsT, ones_sb, start=(st == 0), stop=(st == ST - 1)
                    )
                recip = small.tile([128, 1], F32, tag="recip")
                nc.vector.reciprocal(recip, sS)
                o_sb = opool.tile([128, C], F32, tag="o")
                if fuse_out:
                    nc.vector.tensor_scalar_mul(o_sb, av[:, 0:C], recip)
                else:
                    nc.vector.tensor_scalar_mul(o_sb[:, 0:512], av[:, 0:512], recip)
                    nc.vector.tensor_scalar_mul(
                        o_sb[:, 512:1024], av[:, 512:1024], recip
                    )
                nc.vector.tensor_add(o_sb, o_sb, bvr_sb)
                nc.sync.dma_start(out=out[i * 128 : (i + 1) * 128, :], in_=o_sb)

        for _ in range(repeats):
            if "proj" in phases:
                if exchange:
                    # K first so its exchange overlaps the Q/V projections.
                    wk_sb = load_w_third(C)
                    emit_k_proj_exchange(wk_sb)

                    wq_sb = load_w_third(0)
                    emit_q_proj(wq_sb)

                    wv_sb = load_w_third(2 * C)
                    v_send = emit_v_proj(wv_sb, STH)
                    v_recv = dram.tile([2, STH, 128, C], BF16, tag="v_recv")
                    if not no_cc:
                        nc.gpsimd.collective_compute(
                            "AllGather",
                            mybir.AluOpType.bypass,
                            replica_groups=PAIRS,
                            ins=[v_send.opt()],
                            outs=[v_recv.opt()],
                        )
                    for r in range(2):
                        for st in range(STH):
                            nc.sync.dma_start(
                                out=V_sb[:, r * STH + st, :], in_=v_recv[r, st, :, :]
                            )
                else:
                    wk_sb = load_w_third(C)
                    emit_k_proj_local(wk_sb)
                    wq_sb = load_w_third(0)
                    emit_q_proj(wq_sb)
                    wv_sb = load_w_third(2 * C)
                    emit_v_proj(wv_sb, ST)

            if "scores" in phases:
                emit_scores()
            if "av" in phases:
                emit_av()

    nc.compile()
    return nc


def _get_nc():
    global _NC_CACHE
    if _NC_CACHE is None:
        _NC_CACHE = _build_nc()
    return _NC_CACHE


def _make_in_maps(x, W_qkv, b_qkv):
    W_b16 = np.ascontiguousarray(W_qkv.astype(ml_dtypes.bfloat16))
    # biases laid out [128, CT]: col j holds bias[j*128 : (j+1)*128]
    bq = np.ascontiguousarray(b_qkv[:C].reshape(CT, 128).T.astype(np.float32))
    bk = np.ascontiguousarray(b_qkv[C : 2 * C].reshape(CT, 128).T.astype(np.float32))
    bvr = np.ascontiguousarray(
        np.broadcast_to(b_qkv[2 * C :].astype(np.float32), (128, C))
    )
    in_maps = []
    for core in range(N_CORES):
        b, h = core // 2, core % 2
        xb = x[b]
        if h == 1:  # roll so this core's query half comes first
            xb = np.concatenate([xb[TQ:], xb[:TQ]], axis=0)
        xT = np.ascontiguousarray(xb.T).astype(ml_dtypes.bfloat16)
        in_maps.append({"xT": xT, "W": W_b16, "bq": bq, "bk": bk, "bvr": bvr})
    return in_maps


def _run(x, W_qkv, b_qkv, trace=False, **spmd_kwargs):
    nc = _get_nc()
    in_maps = _make_in_maps(x, W_qkv, b_qkv)
    res = run_bass_kernel_spmd(
        nc, in_maps, list(range(N_CORES)), trace=trace, **spmd_kwargs
    )
    out = np.empty((B, T, C), dtype=np.float32)
    for core in range(N_CORES):
        b, h = core // 2, core % 2
        out[b, h * TQ : (h + 1) * TQ, :] = res.results[core]["out"]
    return out, res


def kernel(x, W_qkv, b_qkv):
    x = np.asarray(x)
    W_qkv = np.asarray(W_qkv)
    b_qkv = np.asarray(b_qkv)
    out, _ = _run(x, W_qkv, b_qkv)
    return out
